# revision 10
# baseline (speedup 1.0000x reference)
"""LinearSplineLayer forward Trainium2 kernel.

Data-parallel over batch across 8 NeuronCores. Per core:
  - x shard [4096, 64] laid out in SBUF as [128 partitions, 32*64].
  - bin index per element on DVE (floor via floored-mod), packed (cdf, slope/64)
    table gathered per element with GPSIMD ap_gather (table replicated on all
    128 partitions; per-16-partition-group index lists come straight from the
    natural element layout).
  - y = (C + dx*S)*2 - 1 on DVE, logdet = sum_d Ln(S) on ACT+DVE.
The tiny pdf-derived tables are precomputed on host (fp32, mirroring the
reference's op order) and shipped as inputs.
"""
import sys

sys.path.insert(0, "/opt/trn_rl_repo")

import numpy as np

import concourse.bass as bass
import concourse.tile as tile
from concourse import mybir
from concourse.bass_utils import run_bass_kernel_spmd
from concourse.vector_clock import ScopedClock

F32 = mybir.dt.float32
I16 = mybir.dt.int16
I8 = mybir.dt.uint8
OP = mybir.AluOpType

B, D, G = 32768, 64, 64
NCORES = 8
BL = B // NCORES            # 4096 rows per core
BT = BL // 128              # 32 column-blocks of 64
FD = BT * D                 # 2048 free dim
NE = (G + 2) * D            # 4224 table entries
CH = 4                      # gather chunks
SC = FD // CH               # 512 idx columns per chunk (num_idxs = 16*SC)

# ---------------------------------------------------------------- tile patch
# This container's walrus rejects >1 sync-wait command on a CTRL_NO (Drain)
# instruction; split the end-of-kernel global-clock waits across drains.
def _drain_and_barrier(self, tick_clock, wait_clock):
    nc = self.nc
    drain_inst = nc.sync.drain()
    wait_clock.add_sem_waits(
        drain_inst.ins, ScopedClock({None: tick_clock.global_clock})
    )
    si = drain_inst.ins.sync_info
    if si is not None and si.on_wait and len(si.on_wait) > 1:
        waits = list(si.on_wait)
        si.on_wait = waits[:1]
        for i in range(1, len(waits)):
            extra = nc.sync.drain()
            esi = extra.ins.sync_info
            if esi is None:
                extra.ins.sync_info = mybir.SyncInfo(
                    on_wait=waits[i : i + 1], on_update=[]
                )
            else:
                esi.on_wait = waits[i : i + 1]
    nc.all_engine_barrier()
    assert self.sems is not None
    popped = nc._tile_sem_poison_stack.pop()
    assert popped is self._sem_poison
    nc.clear_and_free_semaphores(list(self.sems.allocated().values()))
    nc.all_engine_barrier()


tile.TileContext._drain_and_barrier = _drain_and_barrier


def _split_waits(nc):
    """Walrus here allows only one sync-wait command per instruction: hoist
    extra waits onto same-engine NOPs inserted right before the instruction."""
    ctr = [0]
    for f in nc.m.functions:
        for blk in f.blocks:
            new_insts = []
            for ins in blk.instructions:
                si = ins.sync_info
                if si is not None and si.on_wait and len(si.on_wait) > 1:
                    waits = list(si.on_wait)
                    for w in waits[:-1]:
                        nop = mybir.InstNoOp(name=f"waitnop_{ctr[0]}")
                        ctr[0] += 1
                        nop.engine = ins.engine
                        nop.sync_info = mybir.SyncInfo(on_wait=[w], on_update=[])
                        new_insts.append(nop)
                    si.on_wait = waits[-1:]
                new_insts.append(ins)
            blk.instructions[:] = new_insts


# ---------------------------------------------------------------- device code
def _build():
    nc = bass.Bass()
    x_d = nc.dram_tensor("x", [BL, D], F32, kind="ExternalInput")
    tab_d = nc.dram_tensor("tab", [NE * 2], F32, kind="ExternalInput")
    y_d = nc.dram_tensor("y", [BL, D], F32, kind="ExternalOutput")
    ld_d = nc.dram_tensor("ld", [128, BT], F32, kind="ExternalOutput")

    xv = x_d[:, :].rearrange("(bt p) d -> p bt d", p=128)      # [128, 32, 64]
    yv = y_d[:, :].rearrange("(bt p) d -> p bt d", p=128)

    with tile.TileContext(nc) as tc:
        with (
            tc.tile_pool(name="w", bufs=5) as wp,       # rotating f32 temps
            tc.tile_pool(name="keep", bufs=1) as kp,    # long-lived tiles
            tc.tile_pool(name="dstp", bufs=1) as dp,    # gather dst
        ):
            tbl = kp.tile([128, NE * 2], F32, tag="tbl")
            nc.sync.dma_start(tbl, tab_d[:].partition_broadcast(128))
            xt = wp.tile([128, BT, D], F32, tag="w")
            nc.sync.dma_start(xt, xv)

            x2 = xt.rearrange("p bt d -> p (bt d)")
            t = kp.tile([128, FD], F32, tag="t")
            # t = 64*xs = (x + 1)*32  (exact 2^5 scaling of the ref's xs)
            nc.vector.tensor_scalar(t, x2, 1.0, 32.0, OP.add, OP.mult)
            # exact floor: r = RNE-round(t) via 2^23 magic add, then fix ties
            r = wp.tile([128, FD], F32, tag="w")
            nc.vector.tensor_scalar(r, t, 12582912.0, -12582912.0, OP.add, OP.add)
            g = wp.tile([128, FD], F32, tag="w")
            nc.vector.tensor_tensor(g, r, t, OP.is_gt)
            kf = wp.tile([128, FD], F32, tag="w")
            nc.vector.tensor_tensor(kf, r, g, OP.subtract)
            kfc = wp.tile([128, FD], F32, tag="w")
            nc.vector.tensor_scalar(kfc, kf, 0.0, 64.0, OP.max, OP.min)
            dx64 = kp.tile([128, FD], F32, tag="dx64")
            nc.vector.tensor_tensor(dx64, t, kfc, OP.subtract)
            a = wp.tile([128, FD], F32, tag="w")
            nc.vector.tensor_scalar(a, kf, 64.0, 1.0, OP.min, OP.add)
            jf = kp.tile([128, FD], F32, tag="jf")
            nc.vector.tensor_scalar(jf, a, 0.0, None, OP.max)

            # tbl layout [j, d, 2]; per-j views broadcast over the 32 bt blocks
            tblv = tbl.rearrange("p (j d two) -> p j d two", d=D, two=2)
            accC = kp.tile([128, FD], F32, tag="accC")
            accS = kp.tile([128, FD], F32, tag="accS")
            nc.vector.memset(accC, 0.0)
            nc.vector.memset(accS, 1.0)
            accCv = accC.rearrange("p (bt d) -> p bt d", d=D)
            accSv = accS.rearrange("p (bt d) -> p bt d", d=D)
            jfv = jf.rearrange("p (bt d) -> p bt d", d=D)
            for j in range(G + 2):
                mask = wp.tile([128, FD], I8, tag="mask")
                nc.vector.tensor_scalar(mask, jf, float(j), None, OP.is_equal)
                maskv = mask.rearrange("p (bt d) -> p bt d", d=D)
                for w, accv in ((0, accCv), (1, accSv)):
                    base = tblv[:, j, :, w]
                    bview = bass.AP(tensor=base.tensor, offset=base.offset,
                                    ap=[list(base.ap[0]), [0, BT], list(base.ap[1])])
                    nc.vector.copy_predicated(accv, maskv, bview)

            Cl = accC
            Sl = accS
            m = wp.tile([128, FD], F32, tag="w")
            nc.vector.tensor_tensor(m, dx64, Sl, OP.mult)
            yu = wp.tile([128, FD], F32, tag="w")
            nc.vector.tensor_tensor(yu, Cl, m, OP.add)
            yout = wp.tile([128, FD], F32, tag="w")
            nc.vector.tensor_scalar(yout, yu, 2.0, -1.0, OP.mult, OP.add)
            nc.sync.dma_start(yv, yout.rearrange("p (bt d) -> p bt d", d=D))

            ls = wp.tile([128, FD], F32, tag="w")
            nc.scalar.activation(ls, Sl, mybir.ActivationFunctionType.Ln,
                                 scale=64.0)
            ldp = kp.tile([128, BT], F32, tag="ldp")
            nc.vector.reduce_sum(
                out=ldp,
                in_=ls.rearrange("p (bt d) -> p bt d", d=D),
                axis=mybir.AxisListType.X,
            )
            nc.sync.dma_start(ld_d[:, :], ldp)
    _split_waits(nc)
    return nc


_NC = None


def _tables(pdf):
    """fp32 tables mirroring the reference's op order. Returns [NE*2] packed
    (C, S/64) in j-major flat layout e = j*64 + d."""
    pdfm = np.asarray(pdf, dtype=np.float32)[0]          # [D, G]
    mx = pdfm.max(axis=-1, keepdims=True)
    e = np.exp(pdfm - mx, dtype=np.float32)
    p = (e / e.sum(axis=-1, keepdims=True, dtype=np.float32)).astype(np.float32)
    cdf = np.concatenate(
        [np.zeros((D, 1), np.float32), np.cumsum(p, axis=-1, dtype=np.float32)],
        axis=-1,
    ).astype(np.float32)                                  # [D, G+1]
    den = np.float32(np.float32(1.0 / G) + np.float32(1e-7))
    slope = ((cdf[:, 1:] - cdf[:, :-1]) / den).astype(np.float32)   # [D, G]
    ones = np.ones((D, 1), np.float32)
    Ct = np.concatenate([np.zeros((D, 1), np.float32), cdf], axis=-1)  # [D, G+2]
    St = np.concatenate([ones, slope, ones], axis=-1)                  # [D, G+2]
    Sp = (St / np.float32(64.0)).astype(np.float32)
    tabf = np.stack([Ct.T, Sp.T], axis=-1)               # [G+2, D, 2]
    return np.ascontiguousarray(tabf.reshape(-1))


def kernel(x, pdf):
    global _NC
    if _NC is None:
        _NC = _build()
    x = np.ascontiguousarray(np.asarray(x, dtype=np.float32))
    tab = _tables(pdf)
    in_maps = [
        {"x": x[c * BL : (c + 1) * BL], "tab": tab}
        for c in range(NCORES)
    ]
    res = run_bass_kernel_spmd(_NC, in_maps, core_ids=list(range(NCORES)))
    y = np.empty((B, D), np.float32)
    ld = np.empty((B, 1), np.float32)
    for c in range(NCORES):
        y[c * BL : (c + 1) * BL] = res.results[c]["y"]
        # ld result is [128, BT]; row b = bt*128 + p
        ld[c * BL : (c + 1) * BL, 0] = res.results[c]["ld"].T.reshape(-1)
    return y, ld
